# revision 32
# baseline (speedup 1.0000x reference)
"""Trainium2 Bass kernel for nn_AttentionBlock (8-core SPMD, query-row sharded).

Reference (per core, q = 2048 rows of x):
  XQ = x @ Wq; YK = y @ Wk; YV = y @ Wv
  S = (XQ @ YK^T) / 16;  A = (0.1*relu(S) + softmax(S)) / rowsum(...)
  out = A @ YV

Algebra: S has rank <= 7 because S = x @ (Wq @ Wk^T) @ y^T.  With
C = Wq @ Wk^T [256, 7] precomputed, per core:
  P8 = C^T @ x^T                [7, q]      (tiny contraction)
  S^T = y @ P8                  [keys, q]   (rank-7, keys on partitions)
  U = exp(S^T/16), V = 0.1*relu(S^T/16)
  H1 = Y8^T @ U, H2 = Y8^T @ V  [rank8, q]  with Y8 = [y | 1]
  G1 = H1^T @ Wvo8, G2 = H2^T @ Wvo8        (Wvo8 = [[Wv, 0], [0, 1]])
  out = (G1[:, :256]/Z + G2[:, :256]) / D,  Z = G1[:, 256], D = 1 + G2[:, 256]

Device-side structure (per 512-query block):
  - S^T built 3 k-tiles at a time with 3x32-row PE tiling (contraction is
    only 8) into a 3-bank PSUM region [128, 1536]; double buffered.
  - One exp activation per region (scalar engine is the kernel's roofline:
    M*Q/core = 8.4M elems at 1 elem/cycle/lane @ 1.2 GHz ~= 55 us).
  - relu stream split between DVE and ACT (Relu shares exp's table set) to
    balance the two engines; GpSimd cannot read PSUM on TRN2.
  - H1/H2 accumulate into [32, 512] PSUM accumulators (full-array matmuls;
    column-tiled variants trip a walrus codegen bug on partition-offset
    PSUM destinations).
  - Epilogue fused with tensor_scalar / scalar_tensor_tensor; Z and D come
    from the ones-column of Wvo8 (+1 via a rank-1 matmul).
All transposes/layout prep (x^T, y^T strips, Y8, C4, Wvo8) happen on host.
"""

import numpy as np

import concourse.bass as bass
import concourse.mybir as mybir
import concourse.tile as tile
from concourse import bacc
from concourse.bass_utils import run_bass_kernel_spmd

P = 128
N_CORES = 8
N_FULL, M_CTX, SIN, YDIM, SPROJ = 16384, 4096, 256, 7, 256
Q = N_FULL // N_CORES          # 2048 query rows per core
KT = M_CTX // P                # 32 k-tiles
CC = SIN // P                  # 2 contraction chunks for P8
QB = 512                       # q-block width
NQB = Q // QB                  # 4 q-blocks
SCALE = 1.0 / 16.0             # 1/sqrt(SPROJ)
GRP = 3                        # k-tiles per S region (3 PSUM banks)
NG = (KT + GRP - 1) // GRP     # 11 groups (10x3 + 1x2)
R = 32                         # rank dim of the H accumulators
GW = 258                       # G free width: 256 out + Z/D col + pad
RELU_ACT_EVERY = 5             # every 5th group's relu runs on ACT, rest DVE

F32 = mybir.dt.float32
RDT = mybir.dt.float32r


def _grp_w(g):
    return (KT - g * GRP if g == NG - 1 else GRP) * P


def _build():
    nc = bacc.Bacc(
        "TRN2",
        target_bir_lowering=False,
        debug=False,
        num_devices=N_CORES,
    )
    # host-prepped layouts (see kernel()); declared float32r (same bits as
    # f32) so DMA lands directly in matmul-ready tiles without cast copies
    xT_d = nc.dram_tensor("xT", [P, CC, Q], RDT, kind="ExternalInput").ap()
    yT3_d = nc.dram_tensor("yT3", [P, NG, P], RDT, kind="ExternalInput").ap()
    y8_d = nc.dram_tensor("y8", [P, KT, 32], RDT, kind="ExternalInput").ap()
    c4_d = nc.dram_tensor("c4", [P, CC, P], RDT, kind="ExternalInput").ap()
    wvo8_d = nc.dram_tensor("wvo8", [R, GW], RDT, kind="ExternalInput").ap()
    out_d = nc.dram_tensor("out", [Q, SPROJ], F32, kind="ExternalOutput").ap()

    with tile.TileContext(nc) as tc:
        _body(tc, xT_d, yT3_d, y8_d, c4_d, wvo8_d, out_d)
    nc.compile()
    return nc


def _body(tc, xT_d, yT3_d, y8_d, c4_d, wvo8_d, out_d):
    nc = tc.nc
    Exp = mybir.ActivationFunctionType.Exp
    ReluF = mybir.ActivationFunctionType.Relu
    mult = mybir.AluOpType.mult
    add = mybir.AluOpType.add
    amax = mybir.AluOpType.max

    with (
        tc.tile_pool(name="persist", bufs=1) as persist,
        tc.tile_pool(name="spool", bufs=2, space="PSUM") as spool,
        tc.tile_pool(name="hpool", bufs=2, space="PSUM") as hpool,
        tc.tile_pool(name="uv", bufs=3) as uvpool,
        tc.tile_pool(name="p8pool", bufs=2) as p8pool,
        tc.tile_pool(name="hs", bufs=2) as hspool,
        tc.tile_pool(name="epi", bufs=4) as epi,
    ):
        # ---- persistent inputs, DMA'd straight into matmul dtype tiles ----
        c4r = persist.tile([P, CC, P], RDT, tag="c4r")
        nc.sync.dma_start(c4r[:], c4_d)
        yT3r = persist.tile([P, NG, P], RDT, tag="yT3r")
        nc.sync.dma_start(yT3r[:], yT3_d)
        xTr = persist.tile([P, CC, Q], RDT, tag="xTr")
        for c in range(CC):
            nc.sync.dma_start(xTr[:, c, :QB], xT_d[:, c, :QB])

        # remaining inputs, ordered so q-block 0 can start ASAP
        y8r = persist.tile([P, KT, 32], RDT, tag="y8r")
        nc.sync.dma_start(y8r[:], y8_d)
        wvo8r = persist.tile([R, GW], RDT, tag="wvo8r")
        nc.sync.dma_start(wvo8r[:], wvo8_d)
        ones_q = persist.tile([1, P], F32, tag="ones_q")
        nc.vector.memset(ones_q[:], 1.0)
        w1 = persist.tile([1, GW], F32, tag="w1")
        nc.vector.memset(w1[:], 0.0)
        nc.vector.memset(w1[:, SPROJ:SPROJ + 1], 1.0)
        for qb in range(1, NQB):
            for c in range(CC):
                nc.sync.dma_start(
                    xTr[:, c, qb * QB:(qb + 1) * QB],
                    xT_d[:, c, qb * QB:(qb + 1) * QB],
                )

        # warm the ACT exp table while DMAs run
        warm = epi.tile([1, 1], F32, tag="warm")
        nc.vector.memset(warm[:], 0.0)
        nc.scalar.activation(warm[:], warm[:], Exp, scale=1.0)

        # state carried across the flat (qb, g) schedule
        pend = None  # (h_holder, uvs_last, qb) awaiting final H + G + epilogue

        def do_h(hh, u, v, g, qb):
            """H accumulation for group g (reads u/v slices).

            h1/h2 are allocated lazily at g==0 so their pool slots rotate
            AFTER the previous block's G tiles (same pool) — allocating them
            earlier would deadlock the in-order PE queue.
            """
            if g == 0:
                hh.append(hpool.tile([R, QB], F32, tag="h", name=f"h1_{qb}"))
                hh.append(hpool.tile([R, QB], F32, tag="h", name=f"h2_{qb}"))
            h1, h2 = hh
            n = _grp_w(g) // P
            for j in range(n):
                t = g * GRP + j
                nc.tensor.matmul(
                    h1[:], lhsT=y8r[:, t, :], rhs=u[:, j * QB:(j + 1) * QB],
                    start=(t == 0), stop=(t == KT - 1), skip_group_check=True,
                )
            for j in range(n):
                t = g * GRP + j
                nc.tensor.matmul(
                    h2[:], lhsT=y8r[:, t, :], rhs=v[:, j * QB:(j + 1) * QB],
                    start=(t == 0), stop=(t == KT - 1), skip_group_check=True,
                )

        def finish_block(h1, h2, qb):
            """hs copies, G matmuls, epilogue + out DMA for a finished block.

            GpSimd cannot touch PSUM on TRN2, so everything PSUM-sourced is
            on DVE; gpsimd gets the SBUF-only final multiply and the hs2
            ones-row (which makes G2's Z-column equal D = 1 + sum(V)).
            """
            hs1 = hspool.tile([R, QB], RDT, tag="hs1", name=f"hs1_{qb}")
            nc.vector.tensor_copy(hs1[:], h1[:])
            hs2 = hspool.tile([R, QB], RDT, tag="hs2", name=f"hs2_{qb}")
            nc.vector.tensor_copy(hs2[:], h2[:])
            gs = []
            for qc in range(QB // P):
                g1 = hpool.tile([P, QB], F32, tag="h", name=f"g1_{qb}_{qc}")
                nc.tensor.matmul(
                    g1[:, :GW], lhsT=hs1[:, qc * P:(qc + 1) * P],
                    rhs=wvo8r[:], start=True, stop=True,
                    skip_group_check=True,
                )
                g2 = hpool.tile([P, QB], F32, tag="h", name=f"g2_{qb}_{qc}")
                nc.tensor.matmul(
                    g2[:, :GW], lhsT=hs2[:, qc * P:(qc + 1) * P],
                    rhs=wvo8r[:], start=True, stop=False,
                    skip_group_check=True,
                )
                gs.append((g1, g2))
            # +1 of D = 1 + sum(V): rank-1 matmuls adding wvo8's last row
            # (only col 256 nonzero) into each g2; batched to limit PE
            # tiling-mode switches
            for qc in range(QB // P):
                nc.tensor.matmul(
                    gs[qc][1][:, :GW], lhsT=ones_q[:],
                    rhs=w1[:], start=False, stop=True,
                    skip_group_check=True,
                )
            for qc in range(QB // P):
                g1, g2 = gs[qc]
                # out = (g1*zinv + g2)*dinv, with at most one PSUM input per
                # DVE instruction: acc = g1*zinv*dinv, out = g2*dinv + acc
                zinv = epi.tile([P, 1], F32, tag="zinv")
                nc.vector.reciprocal(zinv[:], g1[:, SPROJ:SPROJ + 1])
                dinv = epi.tile([P, 1], F32, tag="dinv")
                nc.vector.reciprocal(dinv[:], g2[:, SPROJ:SPROJ + 1])
                acc = epi.tile([P, SPROJ], F32, tag="acc")
                nc.vector.tensor_scalar(
                    acc[:], g1[:, :SPROJ], zinv[:], dinv[:], mult, mult
                )
                out_t = epi.tile([P, SPROJ], F32, tag="out")
                nc.vector.scalar_tensor_tensor(
                    out_t[:], g2[:, :SPROJ], dinv[:], acc[:], mult, add
                )
                r0 = qb * QB + qc * P
                nc.sync.dma_start(out_d[r0:r0 + P, :], out_t[:])

        for qb in range(NQB):
            q0 = qb * QB
            # P8 for this q-block: [128, 512] with p8 rows replicated at
            # partition offsets {0, 32, 64} (row strips for the S tiling)
            p8ps = spool.tile([P, GRP * QB], F32, tag="s", name=f"p8ps_{qb}")
            for c in range(CC):
                nc.tensor.matmul(
                    p8ps[:, :QB],
                    lhsT=c4r[:, c, :], rhs=xTr[:, c, q0:q0 + QB],
                    start=(c == 0), stop=(c == CC - 1),
                )
            p8r = p8pool.tile([P, QB], RDT, tag="p8r", name=f"p8r_{qb}")
            nc.vector.tensor_copy(p8r[:], p8ps[:, :QB])

            hh = []  # h1/h2, allocated lazily inside do_h
            prev = None  # (u, v, g) awaiting H matmuls
            for g in range(NG):
                w = _grp_w(g)
                sreg = spool.tile([P, GRP * QB], F32, tag="s", name=f"s_{qb}_{g}")
                for j in range(w // P):
                    nc.tensor.matmul(
                        sreg[:, j * QB:(j + 1) * QB],
                        lhsT=yT3r[32 * j:32 * j + 8, g, :],
                        rhs=p8r[32 * j:32 * j + 8, :],
                        start=True, stop=True,
                    )
                wq_ = w * (QB // P)  # region columns = ktiles * 512
                u = uvpool.tile([P, GRP * QB], RDT, tag="u", name=f"u_{qb}_{g}")
                nc.scalar.activation(u[:, :wq_], sreg[:, :wq_], Exp, scale=SCALE)
                v = uvpool.tile([P, GRP * QB], RDT, tag="v", name=f"v_{qb}_{g}")
                # relu stream: DVE mostly, ACT every RELU_ACT_EVERY'th group
                # (Relu shares the exp table set — no table-switch cost)
                if (qb * NG + g) % RELU_ACT_EVERY == RELU_ACT_EVERY - 1:
                    nc.scalar.activation(
                        v[:, :wq_], sreg[:, :wq_], ReluF, scale=0.1 * SCALE
                    )
                else:
                    nc.vector.tensor_scalar(
                        v[:, :wq_], sreg[:, :wq_], 0.1 * SCALE, 0.0, mult, amax
                    )
                if prev is not None:
                    do_h(hh, *prev, qb)
                elif pend is not None:
                    phh, puv, pqb = pend
                    do_h(phh, *puv, pqb)
                    finish_block(phh[0], phh[1], pqb)
                    pend = None
                prev = (u, v, g)
            pend = (hh, prev, qb)

        phh, puv, pqb = pend
        do_h(phh, *puv, pqb)
        finish_block(phh[0], phh[1], pqb)


_NC_CACHE = None


def _host_prep(x, y, Wq, Wk, Wv):
    """Layout-only host prep: shard x (transposed into lhsT chunks) and build
    the small replicated operand layouts the device kernel expects."""
    x = np.asarray(x, dtype=np.float32)
    y = np.asarray(y, dtype=np.float32)
    Wq = np.asarray(Wq, dtype=np.float32)
    Wk = np.asarray(Wk, dtype=np.float32)
    Wv = np.asarray(Wv, dtype=np.float32)

    # C = Wq @ Wk^T, replicated at partition offsets {0, 32, 64} for the
    # row-tiled S matmuls; laid out as lhsT chunks [128, CC, 128]
    C = (Wq @ Wk.T).astype(np.float32)  # [256, 7]
    c4f = np.zeros((P, CC, P), np.float32)
    for c in range(CC):
        for j in range(3):
            c4f[:, c, 32 * j:32 * j + YDIM] = C[c * P:(c + 1) * P, :]

    # y^T strips: group g, slot j -> partitions 32j..32j+6 hold y^T of
    # k-tile 3g+j (keys on the free dim)
    yT = np.zeros((P, NG, P), np.float32)
    for t in range(KT):
        g, j = divmod(t, GRP)
        yT[32 * j:32 * j + YDIM, g, :] = y[t * P:(t + 1) * P, :].T

    # Y8 = [y | 1 | 0pad] per k-tile
    y8f = np.zeros((P, KT, 32), np.float32)
    for t in range(KT):
        y8f[:, t, :YDIM] = y[t * P:(t + 1) * P, :]
    y8f[:, :, YDIM] = 1.0

    # Wvo8 with ones-column producing Z / sum(V)
    # (the +1 of D is added on device via a rank-1 matmul)
    wvo8f = np.zeros((R, GW), np.float32)
    wvo8f[:YDIM, :SPROJ] = Wv
    wvo8f[YDIM, SPROJ] = 1.0

    in_maps = []
    for i in range(N_CORES):
        xc = x[i * Q:(i + 1) * Q]
        xT = np.ascontiguousarray(xc.T.reshape(CC, P, Q).transpose(1, 0, 2))
        in_maps.append(
            {"xT": xT, "yT3": yT, "y8": y8f, "c4": c4f, "wvo8": wvo8f}
        )
    return in_maps


def kernel(x, y, Wq, Wk, Wv):
    global _NC_CACHE
    if _NC_CACHE is None:
        _NC_CACHE = _build()
    nc = _NC_CACHE
    in_maps = _host_prep(x, y, Wq, Wk, Wv)
    res = run_bass_kernel_spmd(nc, in_maps, core_ids=list(range(N_CORES)))
    return np.concatenate([res.results[i]["out"] for i in range(N_CORES)], axis=0)


# revision 53
# speedup vs baseline: 1.4656x; 1.4656x over previous
"""Trainium2 Bass kernel for nn_AttentionBlock (8-core SPMD, query-row sharded).

Reference (per core, q = 2048 rows of x):
  XQ = x @ Wq; YK = y @ Wk; YV = y @ Wv
  S = (XQ @ YK^T) / 16;  A = (0.1*relu(S) + softmax(S)) / rowsum(...)
  out = A @ YV

Algebra: S has rank <= 7 because S = x @ (Wq @ Wk^T) @ y^T.  With
C = Wq @ Wk^T [256, 7] precomputed, per core:
  P8 = C^T @ x^T                [7, q]      (tiny contraction)
  S^T = y @ P8                  [keys, q]   (rank-7, keys on partitions)
  U = exp(S^T/16), V = 0.1*relu(S^T/16)
  H1 = Y8^T @ U, H2 = Y8^T @ V  [rank8, q]  with Y8 = [y | 1]
  G1 = H1^T @ Wvo8, G2 = H2^T @ Wvo8        (Wvo8 = [[Wv, 0], [0, 1]])
  out = (G1[:, :256]/Z + G2[:, :256]) / D,  Z = G1[:, 256], D = 1 + G2[:, 256]

Device-side structure (per 512-query block):
  - S^T built 3 k-tiles at a time with 3x32-row PE tiling (contraction is
    only 8) into a 3-bank PSUM region [128, 1536]; double buffered.
  - One exp activation per region (scalar engine is the kernel's roofline:
    M*Q/core = 8.4M elems at 1 elem/cycle/lane @ 1.2 GHz ~= 55 us).
  - relu stream split between DVE and ACT (Relu shares exp's table set) to
    balance the two engines; GpSimd cannot read PSUM on TRN2.
  - H1/H2 accumulate into [32, 512] PSUM accumulators (full-array matmuls;
    column-tiled variants trip a walrus codegen bug on partition-offset
    PSUM destinations).
  - Epilogue fused with tensor_scalar / scalar_tensor_tensor; Z and D come
    from the ones-column of Wvo8 (+1 via a rank-1 matmul).
All transposes/layout prep (x^T, y^T strips, Y8, C4, Wvo8) happen on host.
"""

import numpy as np

import concourse.bass as bass
import concourse.mybir as mybir
import concourse.tile as tile
from concourse import bacc
from concourse.bass_utils import run_bass_kernel_spmd

P = 128
N_CORES = 8
N_FULL, M_CTX, SIN, YDIM, SPROJ = 16384, 4096, 256, 7, 256
Q = N_FULL // N_CORES          # 2048 query rows per core
KT = M_CTX // P                # 32 k-tiles
CC = SIN // P                  # 2 contraction chunks for P8
QB = 512                       # q-block width
NQB = Q // QB                  # 4 q-blocks
SCALE = 1.0 / 16.0             # 1/sqrt(SPROJ)
GRP = 3                        # k-tiles per S region (3 PSUM banks)
NG = (KT + GRP - 1) // GRP     # 11 groups (10x3 + 1x2)
R = 32                         # rank dim of the H accumulators
GW = 258                       # G free width: 256 out + Z/D col + pad
RELU_ACT_GROUPS = {2, 3}       # per-block groups whose relu runs on ACT
H_LAG = 3                      # H matmuls trail act/relu by this many groups

F32 = mybir.dt.float32
RDT = mybir.dt.float32r


def _grp_w(g):
    return (KT - g * GRP if g == NG - 1 else GRP) * P


def _build():
    nc = bacc.Bacc(
        "TRN2",
        target_bir_lowering=False,
        debug=False,
        num_devices=N_CORES,
    )
    # host-prepped layouts (see kernel()); declared float32r (same bits as
    # f32) so DMA lands directly in matmul-ready tiles without cast copies
    p8_d = nc.dram_tensor("p8", [P, NQB, QB], RDT, kind="ExternalInput").ap()
    yT3_d = nc.dram_tensor("yT3", [P, NG, P], RDT, kind="ExternalInput").ap()
    y8_d = nc.dram_tensor("y8", [P, KT, 32], RDT, kind="ExternalInput").ap()
    wvo8_d = nc.dram_tensor("wvo8", [R, GW], RDT, kind="ExternalInput").ap()
    out_d = nc.dram_tensor("out", [Q, SPROJ], F32, kind="ExternalOutput").ap()

    with tile.TileContext(nc) as tc:
        _body(tc, p8_d, yT3_d, y8_d, wvo8_d, out_d)
    nc.compile()
    return nc


def _body(tc, p8_d, yT3_d, y8_d, wvo8_d, out_d):
    nc = tc.nc
    Exp = mybir.ActivationFunctionType.Exp
    ReluF = mybir.ActivationFunctionType.Relu
    mult = mybir.AluOpType.mult
    add = mybir.AluOpType.add
    amax = mybir.AluOpType.max

    with (
        tc.tile_pool(name="persist", bufs=1) as persist,
        tc.tile_pool(name="spool", bufs=2, space="PSUM") as spool,
        tc.tile_pool(name="hpool", bufs=2, space="PSUM") as hpool,
        tc.tile_pool(name="uv", bufs=5) as uvpool,
        tc.tile_pool(name="hs", bufs=2) as hspool,
        tc.tile_pool(name="epi", bufs=4) as epi,
    ):
        # ---- persistent inputs, DMA'd straight into matmul dtype tiles ----
        # Two HWDGE queues (SP + ACT) run in parallel; ordered so q-block 0
        # unblocks ASAP: S needs p8[qb0] + yT3, H needs y8.
        p8r = persist.tile([P, NQB, QB], RDT, tag="p8r")
        for qb in range(NQB):
            nc.sync.dma_start(p8r[:, qb, :], p8_d[:, qb, :])
        yT3r = persist.tile([P, NG, P], RDT, tag="yT3r")
        nc.scalar.dma_start(yT3r[:], yT3_d)
        y8r = persist.tile([P, KT, 32], RDT, tag="y8r")
        nc.scalar.dma_start(y8r[:], y8_d)
        wvo8r = persist.tile([R, GW], RDT, tag="wvo8r")
        nc.scalar.dma_start(wvo8r[:], wvo8_d)
        ones_q = persist.tile([1, P], F32, tag="ones_q")
        nc.vector.memset(ones_q[:], 1.0)
        w1 = persist.tile([1, GW], F32, tag="w1")
        nc.vector.memset(w1[:], 0.0)
        nc.vector.memset(w1[:, SPROJ:SPROJ + 1], 1.0)

        # warm the ACT exp table while DMAs run
        warm = epi.tile([1, 1], F32, tag="warm")
        nc.vector.memset(warm[:], 0.0)
        nc.scalar.activation(warm[:], warm[:], Exp, scale=1.0)

        # state carried across the flat (qb, g) schedule
        hq = []        # pending (hh, u, v, g, qb) H-work, drained with lag
        epiq = []      # pending per-qchunk epilogues, drained 2/iteration
        hh_by_qb = {}  # qb -> [h1, h2]

        def do_h(hh, u, v, g, qb):
            """H accumulation for group g (reads u/v slices).

            h1/h2 are allocated lazily at g==0 so their pool slots rotate
            AFTER the previous block's G tiles (same pool) — allocating them
            earlier would deadlock the in-order PE queue.
            """
            if g == 0:
                hh.append(hpool.tile([R, QB], F32, tag="h", name=f"h1_{qb}"))
                hh.append(hpool.tile([R, QB], F32, tag="h", name=f"h2_{qb}"))
            h1, h2 = hh
            n = _grp_w(g) // P
            for j in range(n):
                t = g * GRP + j
                nc.tensor.matmul(
                    h1[:], lhsT=y8r[:, t, :], rhs=u[:, j * QB:(j + 1) * QB],
                    start=(t == 0), stop=(t == KT - 1), skip_group_check=True,
                )
            for j in range(n):
                t = g * GRP + j
                nc.tensor.matmul(
                    h2[:], lhsT=y8r[:, t, :], rhs=v[:, j * QB:(j + 1) * QB],
                    start=(t == 0), stop=(t == KT - 1), skip_group_check=True,
                )

        def start_gs(h1, h2, qb):
            """hs copies + G matmuls for a finished block; queue epilogues.

            GpSimd cannot touch PSUM on TRN2, so everything PSUM-sourced is
            on DVE.
            """
            hs1 = hspool.tile([R, QB], RDT, tag="hs1", name=f"hs1_{qb}")
            nc.vector.tensor_copy(hs1[:], h1[:])
            hs2 = hspool.tile([R, QB], RDT, tag="hs2", name=f"hs2_{qb}")
            nc.vector.tensor_copy(hs2[:], h2[:])
            gs = []
            for qc in range(QB // P):
                if qb == NQB - 1 and qc % 2 == 1:
                    # last block: S regions are done — steal a 3-bank spool
                    # tile for g1+g2 to double the G/epilogue pipeline depth
                    # in the exposed tail
                    st = spool.tile(
                        [P, GRP * QB], F32, tag="s", name=f"gx_{qb}_{qc}"
                    )
                    g1, g2 = st[:, :QB], st[:, QB:2 * QB]
                else:
                    g1 = hpool.tile(
                        [P, QB], F32, tag="h", name=f"g1_{qb}_{qc}"
                    )[:]
                    g2 = hpool.tile(
                        [P, QB], F32, tag="h", name=f"g2_{qb}_{qc}"
                    )[:]
                nc.tensor.matmul(
                    g1[:, :GW], lhsT=hs1[:, qc * P:(qc + 1) * P],
                    rhs=wvo8r[:], start=True, stop=True,
                    skip_group_check=True,
                )
                nc.tensor.matmul(
                    g2[:, :GW], lhsT=hs2[:, qc * P:(qc + 1) * P],
                    rhs=wvo8r[:], start=True, stop=False,
                    skip_group_check=True,
                )
                gs.append((g1, g2))
            # +1 of D = 1 + sum(V): rank-1 matmuls adding wvo8's last row
            # (only col 256 nonzero) into each g2; batched to limit PE
            # tiling-mode switches
            for qc in range(QB // P):
                nc.tensor.matmul(
                    gs[qc][1][:, :GW], lhsT=ones_q[:],
                    rhs=w1[:], start=False, stop=True,
                    skip_group_check=True,
                )
            for qc in range(QB // P):
                epiq.append((gs[qc][0], gs[qc][1], qb, qc))

        def do_epi(g1, g2, qb, qc):
            # out = (g1*zinv + g2)*dinv, with at most one PSUM input per
            # DVE instruction: acc = g1*zinv*dinv, out = g2*dinv + acc
            zinv = epi.tile([P, 1], F32, tag="zinv")
            nc.vector.reciprocal(zinv[:], g1[:, SPROJ:SPROJ + 1])
            dinv = epi.tile([P, 1], F32, tag="dinv")
            nc.vector.reciprocal(dinv[:], g2[:, SPROJ:SPROJ + 1])
            acc = epi.tile([P, SPROJ], F32, tag="acc")
            nc.vector.tensor_scalar(
                acc[:], g1[:, :SPROJ], zinv[:], dinv[:], mult, mult
            )
            out_t = epi.tile([P, SPROJ], F32, tag="out")
            nc.vector.scalar_tensor_tensor(
                out_t[:], g2[:, :SPROJ], dinv[:], acc[:], mult, add
            )
            r0 = qb * QB + qc * P
            nc.sync.dma_start(out_d[r0:r0 + P, :], out_t[:])

        def drain(h_lag):
            """Pop pending H-groups down to the lag; a block's G matmuls
            and epilogue run as one batch when its last group pops (the
            h/g tiles share one 2-bank pool, which forces this order)."""
            while len(hq) > h_lag:
                hh, u, v, g, pqb = hq.pop(0)
                do_h(hh, u, v, g, pqb)
                if g == NG - 1:
                    start_gs(hh[0], hh[1], pqb)
                    while epiq:
                        do_epi(*epiq.pop(0))

        for qb in range(NQB):
            hh = hh_by_qb.setdefault(qb, [])
            for g in range(NG):
                w = _grp_w(g)
                sreg = spool.tile([P, GRP * QB], F32, tag="s", name=f"s_{qb}_{g}")
                for j in range(w // P):
                    nc.tensor.matmul(
                        sreg[:, j * QB:(j + 1) * QB],
                        lhsT=yT3r[32 * j:32 * j + 8, g, :],
                        rhs=p8r[32 * j:32 * j + 8, qb, :],
                        start=True, stop=True,
                    )
                wq_ = w * (QB // P)  # region columns = ktiles * 512
                u = uvpool.tile([P, GRP * QB], RDT, tag="u", name=f"u_{qb}_{g}")
                nc.scalar.activation(u[:, :wq_], sreg[:, :wq_], Exp, scale=SCALE)
                v = uvpool.tile([P, GRP * QB], RDT, tag="v", name=f"v_{qb}_{g}")
                # relu stream: DVE mostly; ACT (Relu shares the exp table
                # set) for the groups that overlap the previous block's
                # epilogue burst on DVE — keeps the S pipeline fed
                if g in RELU_ACT_GROUPS or (qb == NQB - 1 and g >= NG - 2):
                    nc.scalar.activation(
                        v[:, :wq_], sreg[:, :wq_], ReluF, scale=0.1 * SCALE
                    )
                else:
                    nc.vector.tensor_scalar(
                        v[:, :wq_], sreg[:, :wq_], 0.1 * SCALE, 0.0, mult, amax
                    )
                hq.append((hh, u, v, g, qb))
                # taper the lag near the end so the final block's H/G/epi
                # tail is not exposed after the last activation
                if qb == NQB - 1 and g >= NG - 3:
                    drain(1)
                else:
                    drain(H_LAG)

        drain(0)


_NC_CACHE = None


def _host_prep(x, y, Wq, Wk, Wv):
    """Host prep: shard x and fold it through the rank-7 bottleneck
    P8 = x @ (Wq @ Wk^T) (tiny vs the on-device N*M work), plus the small
    replicated operand layouts the device kernel expects."""
    x = np.asarray(x, dtype=np.float32)
    y = np.asarray(y, dtype=np.float32)
    Wq = np.asarray(Wq, dtype=np.float32)
    Wk = np.asarray(Wk, dtype=np.float32)
    Wv = np.asarray(Wv, dtype=np.float32)

    # P8 = x @ (Wq @ Wk^T): [N, 7]; per core laid out [128, NQB, 512] with
    # rows replicated at partition offsets {0, 32, 64} (row strips for the
    # 3x32-row-tiled S matmuls)
    C = (Wq @ Wk.T).astype(np.float32)  # [256, 7]
    P8 = (x @ C).astype(np.float32)     # [N, 7]

    # y^T strips: group g, slot j -> partitions 32j..32j+6 hold y^T of
    # k-tile 3g+j (keys on the free dim)
    yT = np.zeros((P, NG, P), np.float32)
    for t in range(KT):
        g, j = divmod(t, GRP)
        yT[32 * j:32 * j + YDIM, g, :] = y[t * P:(t + 1) * P, :].T

    # Y8 = [y | 1 | 0pad] per k-tile
    y8f = np.zeros((P, KT, 32), np.float32)
    for t in range(KT):
        y8f[:, t, :YDIM] = y[t * P:(t + 1) * P, :]
    y8f[:, :, YDIM] = 1.0

    # Wvo8 with ones-column producing Z / sum(V)
    # (the +1 of D is added on device via a rank-1 matmul)
    wvo8f = np.zeros((R, GW), np.float32)
    wvo8f[:YDIM, :SPROJ] = Wv
    wvo8f[YDIM, SPROJ] = 1.0

    in_maps = []
    for i in range(N_CORES):
        pc = P8[i * Q:(i + 1) * Q].T.reshape(YDIM, NQB, QB)  # [7, NQB, 512]
        p8f = np.zeros((P, NQB, QB), np.float32)
        for j in range(3):
            p8f[32 * j:32 * j + YDIM] = pc
        in_maps.append({"p8": p8f, "yT3": yT, "y8": y8f, "wvo8": wvo8f})
    return in_maps


def kernel(x, y, Wq, Wk, Wv):
    global _NC_CACHE
    if _NC_CACHE is None:
        _NC_CACHE = _build()
    nc = _NC_CACHE
    in_maps = _host_prep(x, y, Wq, Wk, Wv)
    res = run_bass_kernel_spmd(nc, in_maps, core_ids=list(range(N_CORES)))
    return np.concatenate([res.results[i]["out"] for i in range(N_CORES)], axis=0)
